# revision 3
# baseline (speedup 1.0000x reference)
"""Bezier-stroke rasterizer (AIR/Guide-style) as a Trainium2 Bass/Tile kernel.

Math per (batch, stroke): control points -> Bezier curve -> gaussian blob
rasterization summed along the curve -> presence gating -> max-norm ->
tanh-norm -> sum over strokes -> tanh-norm.

Factorization: exp(-inv*(dy^2+dx^2)) = ey[t,y]*ex[t,x], so the raster is
S = ey^T @ ex contracted over curve samples t on the PE partition dim.

Key performance structure vs the naive version:
- T=128 curve samples (one PE chunk) with Euler-Maclaurin endpoint weights
  folded into the Exp activation bias, matching the 500-sample reference
  sum to ~1e-3 (the maxnorm cancels the sample-density factor).
- dx/dy via a single fp32r matmul per quarter (448 cols -> 1 cycle/row).
- E matrices in fp16: stroke matmuls run at 1 cycle/row.
- Max-norm uses PE transposes (no DRAM round trips); per-stroke scale is
  applied by one broadcast DVE multiply; k-sum runs as PSUM-accumulated
  matmuls against identity slices (no DVE reduction).
- Batch halves (4+4) pipeline through the whole epilogue independently.

Sharding: pure data parallel, 8 batches per core across 8 NeuronCores.
"""

import sys
import numpy as np
from math import comb, tanh, log

sys.path.insert(0, "/opt/trn_rl_repo")

from concourse import bass, bacc, tile, mybir, bass_isa  # noqa: E402
from concourse.bass_utils import run_bass_kernel_spmd  # noqa: E402

BS, K, PTS, RES = 64, 4, 5, 28
T = 128                     # curve samples (contraction dim of stroke matmul)
REF_STEPS = 500             # reference's sample count (for endpoint weights)
NCORES = 8
BL = BS // NCORES           # local batches per core = 8
NPAIR = BL * K              # (batch, stroke) pairs per core = 32
W = NPAIR * RES             # 896 columns per coordinate block
Q = W // 2                  # 448 = one PSUM-bank-sized quarter
G = 4 * RES                 # 112 rows per batch group (4 strokes x 28)
EPS = 1e-6
F32 = mybir.dt.float32
F32R = mybir.dt.float32r
F16 = mybir.dt.float16
AF = mybir.ActivationFunctionType
ALU = mybir.AluOpType
AX = mybir.AxisListType


def _host_consts():
    t = np.linspace(0.0, 1.0, T, dtype=np.float32)[:, None]
    i = np.arange(PTS, dtype=np.float32)[None, :]
    binom = np.array([comb(PTS - 1, j) for j in range(PTS)], dtype=np.float32)[None, :]
    basis = binom * (t**i) * ((1.0 - t) ** (PTS - 1 - i))        # [T, 5]
    grid = np.linspace(0.0, 1.0, RES, dtype=np.float32)          # [28]

    # endpoint weights: the reference sums 500 samples; a T-sample sum
    # underweights interior vs endpoints by the density ratio. w folds the
    # Euler-Maclaurin endpoint correction in; applied as ln(w) bias on ey.
    c = (REF_STEPS - 1) / (T - 1)
    w_end = (c + 1.0) / (2.0 * c)
    lnw = np.zeros((T, 1), np.float32)
    lnw[0, 0] = lnw[-1, 0] = log(w_end)

    ident = np.eye(G, dtype=np.float16)                          # [112, 112]
    return basis, grid, lnw, ident


def _build_program(sigma, slope_strk, slope):
    inv = 1.0 / (2.0 * sigma * sigma)
    post1 = 1.0 / tanh(slope_strk)
    post2 = 1.0 / tanh(slope)

    nc = bacc.Bacc(None, target_bir_lowering=False)

    rhs_d = nc.dram_tensor("rhs6", [6, 128 + 2 * W], F32R, kind="ExternalInput")
    cb_d = nc.dram_tensor("cblob", [T, 33], F32, kind="ExternalInput")
    fb_d = nc.dram_tensor("fblob", [G, G], F16, kind="ExternalInput")
    out_d = nc.dram_tensor("out", [BL, RES, RES], F32, kind="ExternalOutput")

    with tile.TileContext(nc) as tc:
        with (
            tc.tile_pool(name="const", bufs=1) as cpool,
            tc.tile_pool(name="work", bufs=1) as wpool,
            tc.tile_pool(name="dxp", bufs=4, space="PSUM") as dxpool,
            tc.tile_pool(name="sp", bufs=2, space="PSUM") as spool,
        ):
            # ---- inputs / constants ----
            rhs6 = cpool.tile([6, 128 + 2 * W], F32R)
            nc.sync.dma_start(rhs6[:], rhs_d[:])
            cblob = cpool.tile([T, 33], F32)
            nc.scalar.dma_start(cblob[:], cb_d[:])
            fblob = cpool.tile([G, G], F16)
            nc.scalar.dma_start(fblob[:], fb_d[:])

            lnw = cblob[:, 0:1]
            ident = fblob[:, 0:G]
            basT6 = rhs6[:, 0:128]

            # ---- dx/dy quarters: one fp32r matmul each ----
            # quarter q: 0 = x half0, 1 = y half0, 2 = x half1, 3 = y half1
            qoff = [128, 128 + W, 128 + Q, 128 + W + Q]
            dxp = []
            for q in range(4):
                p = dxpool.tile([T, Q], F32, tag="dx", name=f"dxp{q}")
                dxp.append(p)
                nc.tensor.matmul(
                    p[:], basT6, rhs6[:, qoff[q] : qoff[q] + Q],
                    start=True, stop=True,
                )

            # ---- square + exp (fp16 out). Act owns h0 squares; h1
            # squares run on DVE via SBUF staging so Act's serial chain
            # shortens and strokes-h1 unblocks earlier.
            E = wpool.tile([T, 2 * W], F16)
            dxs = wpool.tile([T, 2 * Q], F16)
            eoff = [0, W, Q, W + Q]
            nc.scalar.activation(dxp[0][:], dxp[0][:], AF.Square)
            nc.scalar.activation(dxp[1][:], dxp[1][:], AF.Square)
            nc.scalar.activation(
                E[:, eoff[0] : eoff[0] + Q], dxp[0][:], AF.Exp, scale=-inv)
            nc.scalar.activation(
                E[:, eoff[1] : eoff[1] + Q], dxp[1][:], AF.Exp, scale=-inv,
                bias=lnw)
            nc.vector.tensor_copy(dxs[:, 0:Q], dxp[2][:])
            nc.vector.tensor_tensor(
                dxs[:, 0:Q], dxs[:, 0:Q], dxs[:, 0:Q], op=ALU.mult)
            nc.vector.tensor_copy(dxs[:, Q : 2 * Q], dxp[3][:])
            nc.vector.tensor_tensor(
                dxs[:, Q : 2 * Q], dxs[:, Q : 2 * Q], dxs[:, Q : 2 * Q],
                op=ALU.mult)
            nc.scalar.activation(
                E[:, eoff[2] : eoff[2] + Q], dxs[:, 0:Q], AF.Exp, scale=-inv)
            nc.scalar.activation(
                E[:, eoff[3] : eoff[3] + Q], dxs[:, Q : 2 * Q], AF.Exp,
                scale=-inv, bias=lnw)

            # ---- stroke matmuls: S_h[(k,y),(g,k',x)] = ey^T @ ex ----
            S = []
            for h in range(2):
                sh = spool.tile([G, 4 * G], F32, tag="S")
                S.append(sh)
                for gg in range(4):
                    g = 4 * h + gg
                    nc.tensor.matmul(
                        sh[:, G * gg : G * (gg + 1)],
                        E[:, W + G * g : W + G * (g + 1)],
                        E[:, G * g : G * (g + 1)],
                        start=True, stop=True,
                    )

            # ---- epilogue per batch-half ----
            # maxnorm scale: x-max -> +EP (diag +eps/zp, off-diag -BIG: folds
            # mask and eps; max is monotonic) -> cross-partition max (gpsimd
            # ucode all-reduce: every partition gets m_{g,j}) -> reciprocal.
            # Then one broadcast multiply gates S, tanh folds slope_strk.
            BM = [wpool.tile([G, 16], F32, tag=f"bm{h}", name=f"BM{h}") for h in range(2)]
            Tpre = [wpool.tile([G, 4 * G], F32, tag=f"tp{h}", name=f"Tpre{h}") for h in range(2)]
            Tb = [wpool.tile([G, 4 * G], F16, tag=f"tb{h}", name=f"Tb{h}") for h in range(2)]
            at = [wpool.tile([RES, G], F32, tag=f"at{h}", name=f"at{h}") for h in range(2)]
            img = [None, None]

            def bmax(h):
                nc.vector.reduce_max(
                    BM[h][:].rearrange("p (g j) -> p g j", j=K),
                    S[h][:].rearrange("p (g j x) -> p g j x", j=K, x=RES),
                    axis=AX.X,
                )
                nc.vector.tensor_tensor(
                    BM[h][:], BM[h][:], cblob[0:G, 1 + 16 * h : 17 + 16 * h],
                    op=ALU.add,
                )

            def rmax(h):
                nc.gpsimd.partition_all_reduce(
                    BM[h][:], BM[h][:], G, bass_isa.ReduceOp.max)

            def gate(h, parts=(4,)):
                nc.vector.reciprocal(BM[h][:], BM[h][:])
                # h0's gate may split so it can fill DVE gaps between the
                # h1 chain's ops instead of delaying them.
                bounds = [0]
                for p in parts:
                    bounds.append(bounds[-1] + p)
                for s in range(len(parts)):
                    cs, ce = bounds[s], bounds[s + 1]
                    step = parts[s]
                    nc.vector.tensor_tensor(
                        Tpre[h][:, G * cs : G * ce].rearrange(
                            "p (g j x) -> p g j x", j=K, x=RES),
                        S[h][:, G * cs : G * ce].rearrange(
                            "p (g j x) -> p g j x", j=K, x=RES),
                        BM[h][:, 4 * cs : 4 * ce].rearrange(
                            "p (g j) -> p g j", j=K)[
                            :, :, :, None].broadcast_to([G, step, K, RES]),
                        op=ALU.mult,
                    )

            def tanh_ksum(h):
                nc.scalar.activation(
                    Tb[h][:], Tpre[h][:], AF.Tanh, scale=float(slope_strk))
                img[h] = dxpool.tile([RES, G], F32, tag="dx", name=f"img{h}")
                for j in range(K):
                    nc.tensor.matmul(
                        img[h][:],
                        ident[:, RES * j : RES * (j + 1)],
                        Tb[h][:].rearrange(
                            "p (g j x) -> p g j x", j=K, x=RES)[:, :, j : j + 1, :],
                        start=(j == 0), stop=(j == K - 1),
                    )

            def finish(h):
                nc.scalar.activation(
                    at[h][:], img[h][:], AF.Tanh, scale=float(slope) * post1)
                nc.vector.tensor_scalar_mul(at[h][:], at[h][:], post2)
                eng = nc.scalar if h == 0 else nc.sync
                eng.dma_start(
                    out_d[4 * h : 4 * h + 4].rearrange("b y x -> y b x"),
                    at[h][:].rearrange("p (b x) -> p b x", x=RES),
                )

            bmax(0)
            rmax(0)
            bmax(1)
            gate(0, parts=(2, 2))
            rmax(1)
            gate(1)
            tanh_ksum(0)
            tanh_ksum(1)
            finish(0)
            finish(1)

    nc.compile()
    return nc


_CACHE = {}


def _get_program(sigma, slope_strk, slope):
    key = (float(sigma), float(slope_strk), float(slope))
    if key not in _CACHE:
        _CACHE[key] = _build_program(*key)
    return _CACHE[key]


def _host_inputs(z_pres, z_what, z_where):
    basis, grid, lnw, ident = _host_consts()
    fblob = ident

    in_maps = []
    for c in range(NCORES):
        sl = slice(c * BL, (c + 1) * BL)
        zw = z_what[sl].reshape(NPAIR, PTS, 2)                   # [32, 5, 2]
        zwh = z_where[sl].reshape(NPAIR, 3)
        zp = z_pres[sl]                                          # [8, 4]
        s = zwh[:, 0:1]
        pts = zw * s[:, :, None] + zwh[:, None, 1:3]
        ptsx = pts[:, :, 0]                                      # [32, 5]
        ptsy = pts[:, :, 1]

        rhs6 = np.zeros((6, 128 + 2 * W), np.float32)
        rhs6[:5, 0:128] = basis.T
        rhs6[5, 0:128] = 1.0
        for blk, p5 in ((0, ptsx), (1, ptsy)):
            off = 128 + blk * W
            rhs6[:5, off : off + W] = np.repeat(p5.T, RES, axis=1)
            rhs6[5, off : off + W] = -np.tile(grid, NPAIR)

        cblob = np.zeros((T, 33), np.float32)
        cblob[:, 0:1] = lnw
        # EP_h[(k,y),(g,j)]: +eps/zp on diagonal stroke blocks (j==k), -BIG
        # off-diagonal -- one add replaces the mask-mult and the eps-add.
        epszp = EPS / np.maximum(zp, 1e-37)                      # [8, 4]
        kidx = np.arange(G) // RES                               # [112]
        diag = (kidx[:, None] == np.arange(K)[None, :])          # [112, 4]
        for h in range(2):
            ep = np.where(diag[:, None, :],
                          epszp[4 * h : 4 * h + 4][None, :, :],
                          np.float32(-1e30))                     # [112, 4, 4]
            cblob[0:G, 1 + 16 * h : 17 + 16 * h] = ep.reshape(G, 16)

        in_maps.append({
            "rhs6": np.ascontiguousarray(rhs6),
            "cblob": np.ascontiguousarray(cblob),
            "fblob": np.ascontiguousarray(fblob),
        })
    return in_maps


def kernel(z_pres, z_what, z_where, sigma, slope_strk, slope):
    z_pres = np.asarray(z_pres, np.float32)
    z_what = np.asarray(z_what, np.float32)
    z_where = np.asarray(z_where, np.float32)
    nc = _get_program(float(sigma), float(slope_strk), float(slope))
    in_maps = _host_inputs(z_pres, z_what, z_where)
    res = run_bass_kernel_spmd(nc, in_maps, core_ids=list(range(NCORES)))
    out = np.concatenate([r["out"] for r in res.results], axis=0)
    return out[:, None].astype(np.float32)


# revision 4
# speedup vs baseline: 1.0114x; 1.0114x over previous
"""Bezier-stroke rasterizer (AIR/Guide-style) as a Trainium2 Bass/Tile kernel.

Math per (batch, stroke): control points -> Bezier curve -> gaussian blob
rasterization summed along the curve -> presence gating -> max-norm ->
tanh-norm -> sum over strokes -> tanh-norm.

Factorization: exp(-inv*(dy^2+dx^2)) = ey[t,y]*ex[t,x], so the raster is
S = ey^T @ ex contracted over curve samples t on the PE partition dim.

Key performance structure vs the naive version:
- T=128 curve samples (one PE chunk) with Euler-Maclaurin endpoint weights
  folded into the Exp activation bias, matching the 500-sample reference
  sum to ~1e-3 (the maxnorm cancels the sample-density factor).
- dx/dy via a single fp32r matmul per quarter (448 cols -> 1 cycle/row).
- E matrices in fp16: stroke matmuls run at 1 cycle/row.
- Max-norm: per-row x-max, one add of a host-built tensor (+eps/z_pres on
  diagonal stroke blocks, -BIG off-diagonal -- folds the block mask and the
  maxnorm epsilon, since max is monotonic), then a gpsimd ucode
  partition_all_reduce(max) so every partition holds the per-stroke max
  (no transposes, no DRAM round trips), reciprocal, one broadcast multiply.
- k-sum runs as PSUM-accumulated matmuls against identity slices.
- Batch halves (4+4) pipeline through the whole epilogue independently;
  the first half's gate is split so it fills DVE gaps in the second
  half's critical chain.

Sharding: pure data parallel, 8 batches per core across 8 NeuronCores.
"""

import sys
import numpy as np
from math import comb, tanh, log

sys.path.insert(0, "/opt/trn_rl_repo")

from concourse import bass, bacc, tile, mybir, bass_isa  # noqa: E402
from concourse.bass_utils import run_bass_kernel_spmd  # noqa: E402

BS, K, PTS, RES = 64, 4, 5, 28
T = 128                     # curve samples (contraction dim of stroke matmul)
REF_STEPS = 500             # reference's sample count (for endpoint weights)
NCORES = 8
BL = BS // NCORES           # local batches per core = 8
NPAIR = BL * K              # (batch, stroke) pairs per core = 32
W = NPAIR * RES             # 896 columns per coordinate block
Q = W // 2                  # 448 = one PSUM-bank-sized quarter
G = 4 * RES                 # 112 rows per batch group (4 strokes x 28)
EPS = 1e-6
F32 = mybir.dt.float32
F32R = mybir.dt.float32r
F16 = mybir.dt.float16
AF = mybir.ActivationFunctionType
ALU = mybir.AluOpType
AX = mybir.AxisListType


def _host_consts():
    t = np.linspace(0.0, 1.0, T, dtype=np.float32)[:, None]
    i = np.arange(PTS, dtype=np.float32)[None, :]
    binom = np.array([comb(PTS - 1, j) for j in range(PTS)], dtype=np.float32)[None, :]
    basis = binom * (t**i) * ((1.0 - t) ** (PTS - 1 - i))        # [T, 5]
    grid = np.linspace(0.0, 1.0, RES, dtype=np.float32)          # [28]

    # endpoint weights: the reference sums 500 samples; a T-sample sum
    # underweights interior vs endpoints by the density ratio. w folds the
    # Euler-Maclaurin endpoint correction in; applied as ln(w) bias on ey.
    c = (REF_STEPS - 1) / (T - 1)
    w_end = (c + 1.0) / (2.0 * c)
    lnw = np.zeros((T, 1), np.float32)
    lnw[0, 0] = lnw[-1, 0] = log(w_end)

    ident = np.eye(G, dtype=np.float16)                          # [112, 112]
    return basis, grid, lnw, ident


def _build_program(sigma, slope_strk, slope):
    inv = 1.0 / (2.0 * sigma * sigma)
    post1 = 1.0 / tanh(slope_strk)
    post2 = 1.0 / tanh(slope)

    nc = bacc.Bacc(None, target_bir_lowering=False)

    rhs_d = nc.dram_tensor("rhs6", [6, 128 + 2 * W], F32R, kind="ExternalInput")
    cb_d = nc.dram_tensor("cblob", [T, 33], F32, kind="ExternalInput")
    fb_d = nc.dram_tensor("fblob", [G, G], F16, kind="ExternalInput")
    out_d = nc.dram_tensor("out", [BL, RES, RES], F32, kind="ExternalOutput")

    with tile.TileContext(nc) as tc:
        with (
            tc.tile_pool(name="const", bufs=1) as cpool,
            tc.tile_pool(name="work", bufs=1) as wpool,
            tc.tile_pool(name="dxp", bufs=4, space="PSUM") as dxpool,
            tc.tile_pool(name="sp", bufs=2, space="PSUM") as spool,
        ):
            # ---- inputs / constants ----
            rhs6 = cpool.tile([6, 128 + 2 * W], F32R)
            nc.sync.dma_start(rhs6[:], rhs_d[:])
            cblob = cpool.tile([T, 33], F32)
            nc.scalar.dma_start(cblob[:], cb_d[:])
            fblob = cpool.tile([G, G], F16)
            nc.scalar.dma_start(fblob[:], fb_d[:])

            lnw = cblob[:, 0:1]
            ident = fblob[:, 0:G]
            basT6 = rhs6[:, 0:128]

            # ---- dx/dy quarters: one fp32r matmul each ----
            # quarter q: 0 = x half0, 1 = y half0, 2 = x half1, 3 = y half1
            qoff = [128, 128 + W, 128 + Q, 128 + W + Q]
            dxp = []
            for q in range(4):
                p = dxpool.tile([T, Q], F32, tag="dx", name=f"dxp{q}")
                dxp.append(p)
                nc.tensor.matmul(
                    p[:], basT6, rhs6[:, qoff[q] : qoff[q] + Q],
                    start=True, stop=True,
                )

            # ---- square + exp (fp16 out). Act owns h0 squares; h1
            # squares run on DVE via SBUF staging so Act's serial chain
            # shortens and strokes-h1 unblocks earlier.
            E = wpool.tile([T, 2 * W], F16)
            dxs = wpool.tile([T, 2 * Q], F16)
            eoff = [0, W, Q, W + Q]
            nc.scalar.activation(dxp[0][:], dxp[0][:], AF.Square)
            nc.scalar.activation(dxp[1][:], dxp[1][:], AF.Square)
            nc.scalar.activation(
                E[:, eoff[0] : eoff[0] + Q], dxp[0][:], AF.Exp, scale=-inv)
            nc.scalar.activation(
                E[:, eoff[1] : eoff[1] + Q], dxp[1][:], AF.Exp, scale=-inv,
                bias=lnw)
            nc.vector.tensor_copy(dxs[:, 0:Q], dxp[2][:])
            nc.vector.tensor_tensor(
                dxs[:, 0:Q], dxs[:, 0:Q], dxs[:, 0:Q], op=ALU.mult)
            nc.vector.tensor_copy(dxs[:, Q : 2 * Q], dxp[3][:])
            nc.vector.tensor_tensor(
                dxs[:, Q : 2 * Q], dxs[:, Q : 2 * Q], dxs[:, Q : 2 * Q],
                op=ALU.mult)
            nc.scalar.activation(
                E[:, eoff[2] : eoff[2] + Q], dxs[:, 0:Q], AF.Exp, scale=-inv)
            nc.scalar.activation(
                E[:, eoff[3] : eoff[3] + Q], dxs[:, Q : 2 * Q], AF.Exp,
                scale=-inv, bias=lnw)

            # ---- stroke matmuls: S_h[(k,y),(g,k',x)] = ey^T @ ex ----
            S = []
            for h in range(2):
                sh = spool.tile([G, 4 * G], F32, tag="S")
                S.append(sh)
                for gg in range(4):
                    g = 4 * h + gg
                    nc.tensor.matmul(
                        sh[:, G * gg : G * (gg + 1)],
                        E[:, W + G * g : W + G * (g + 1)],
                        E[:, G * g : G * (g + 1)],
                        start=True, stop=True,
                    )

            # ---- epilogue per batch-half ----
            # maxnorm scale: x-max -> +EP (diag +eps/zp, off-diag -BIG: folds
            # mask and eps; max is monotonic) -> cross-partition max (gpsimd
            # ucode all-reduce: every partition gets m_{g,j}) -> reciprocal.
            # Then one broadcast multiply gates S, tanh folds slope_strk.
            BM = [wpool.tile([G, 16], F32, tag=f"bm{h}", name=f"BM{h}") for h in range(2)]
            Tpre = [wpool.tile([G, 4 * G], F32, tag=f"tp{h}", name=f"Tpre{h}") for h in range(2)]
            Tb = [wpool.tile([G, 4 * G], F16, tag=f"tb{h}", name=f"Tb{h}") for h in range(2)]
            at = [wpool.tile([RES, G], F32, tag=f"at{h}", name=f"at{h}") for h in range(2)]
            img = [None, None]

            def bmax(h):
                nc.vector.reduce_max(
                    BM[h][:].rearrange("p (g j) -> p g j", j=K),
                    S[h][:].rearrange("p (g j x) -> p g j x", j=K, x=RES),
                    axis=AX.X,
                )
                nc.vector.tensor_tensor(
                    BM[h][:], BM[h][:], cblob[0:G, 1 + 16 * h : 17 + 16 * h],
                    op=ALU.add,
                )

            def rmax(h):
                nc.gpsimd.partition_all_reduce(
                    BM[h][:], BM[h][:], G, bass_isa.ReduceOp.max)

            def gate(h, parts=(4,)):
                nc.vector.reciprocal(BM[h][:], BM[h][:])
                # h0's gate may split so it can fill DVE gaps between the
                # h1 chain's ops instead of delaying them.
                bounds = [0]
                for p in parts:
                    bounds.append(bounds[-1] + p)
                for s in range(len(parts)):
                    cs, ce = bounds[s], bounds[s + 1]
                    step = parts[s]
                    nc.vector.tensor_tensor(
                        Tpre[h][:, G * cs : G * ce].rearrange(
                            "p (g j x) -> p g j x", j=K, x=RES),
                        S[h][:, G * cs : G * ce].rearrange(
                            "p (g j x) -> p g j x", j=K, x=RES),
                        BM[h][:, 4 * cs : 4 * ce].rearrange(
                            "p (g j) -> p g j", j=K)[
                            :, :, :, None].broadcast_to([G, step, K, RES]),
                        op=ALU.mult,
                    )

            def tanh_ksum(h):
                nc.scalar.activation(
                    Tb[h][:], Tpre[h][:], AF.Tanh, scale=float(slope_strk))
                img[h] = dxpool.tile([RES, G], F32, tag="dx", name=f"img{h}")
                for j in range(K):
                    nc.tensor.matmul(
                        img[h][:],
                        ident[:, RES * j : RES * (j + 1)],
                        Tb[h][:].rearrange(
                            "p (g j x) -> p g j x", j=K, x=RES)[:, :, j : j + 1, :],
                        start=(j == 0), stop=(j == K - 1),
                    )

            def finish(h):
                nc.scalar.activation(
                    at[h][:], img[h][:], AF.Tanh, scale=float(slope) * post1)
                nc.vector.tensor_scalar_mul(at[h][:], at[h][:], post2)
                eng = nc.scalar if h == 0 else nc.sync
                eng.dma_start(
                    out_d[4 * h : 4 * h + 4].rearrange("b y x -> y b x"),
                    at[h][:].rearrange("p (b x) -> p b x", x=RES),
                )

            bmax(0)
            rmax(0)
            bmax(1)
            gate(0, parts=(2, 2))
            rmax(1)
            gate(1)
            tanh_ksum(0)
            tanh_ksum(1)
            finish(0)
            finish(1)

    nc.compile()
    return nc


_CACHE = {}


def _get_program(sigma, slope_strk, slope):
    key = (float(sigma), float(slope_strk), float(slope))
    if key not in _CACHE:
        _CACHE[key] = _build_program(*key)
    return _CACHE[key]


def _host_inputs(z_pres, z_what, z_where):
    basis, grid, lnw, ident = _host_consts()
    fblob = ident

    in_maps = []
    for c in range(NCORES):
        sl = slice(c * BL, (c + 1) * BL)
        zw = z_what[sl].reshape(NPAIR, PTS, 2)                   # [32, 5, 2]
        zwh = z_where[sl].reshape(NPAIR, 3)
        zp = z_pres[sl]                                          # [8, 4]
        s = zwh[:, 0:1]
        pts = zw * s[:, :, None] + zwh[:, None, 1:3]
        ptsx = pts[:, :, 0]                                      # [32, 5]
        ptsy = pts[:, :, 1]

        rhs6 = np.zeros((6, 128 + 2 * W), np.float32)
        rhs6[:5, 0:128] = basis.T
        rhs6[5, 0:128] = 1.0
        for blk, p5 in ((0, ptsx), (1, ptsy)):
            off = 128 + blk * W
            rhs6[:5, off : off + W] = np.repeat(p5.T, RES, axis=1)
            rhs6[5, off : off + W] = -np.tile(grid, NPAIR)

        cblob = np.zeros((T, 33), np.float32)
        cblob[:, 0:1] = lnw
        # EP_h[(k,y),(g,j)]: +eps/zp on diagonal stroke blocks (j==k), -BIG
        # off-diagonal -- one add replaces the mask-mult and the eps-add.
        epszp = EPS / np.maximum(zp, 1e-37)                      # [8, 4]
        kidx = np.arange(G) // RES                               # [112]
        diag = (kidx[:, None] == np.arange(K)[None, :])          # [112, 4]
        for h in range(2):
            ep = np.where(diag[:, None, :],
                          epszp[4 * h : 4 * h + 4][None, :, :],
                          np.float32(-1e30))                     # [112, 4, 4]
            cblob[0:G, 1 + 16 * h : 17 + 16 * h] = ep.reshape(G, 16)

        in_maps.append({
            "rhs6": np.ascontiguousarray(rhs6),
            "cblob": np.ascontiguousarray(cblob),
            "fblob": np.ascontiguousarray(fblob),
        })
    return in_maps


def kernel(z_pres, z_what, z_where, sigma, slope_strk, slope):
    z_pres = np.asarray(z_pres, np.float32)
    z_what = np.asarray(z_what, np.float32)
    z_where = np.asarray(z_where, np.float32)
    nc = _get_program(float(sigma), float(slope_strk), float(slope))
    in_maps = _host_inputs(z_pres, z_what, z_where)
    res = run_bass_kernel_spmd(nc, in_maps, core_ids=list(range(NCORES)))
    out = np.concatenate([r["out"] for r in res.results], axis=0)
    return out[:, None].astype(np.float32)
